# revision 28
# baseline (speedup 1.0000x reference)
"""Trainium2 Bass kernel for nn_CrossModalFusion (single-head cross attention).

Per-batch-element cross attention, data-parallel over B=8 across 8 NeuronCores.

Per core (T=2048, D_RGB=400, D_POSE=256, H=512):
    q = rgb @ Wq + bq ; k = pose @ Wk + bk ; v = pose @ Wv
    S = q @ k.T / sqrt(H) ; A = exp(S) (no max-sub needed; scores are O(1))
    y = rgb + bp + bv@Wp + (A @ v) @ Wp / rowsum(A)

Layout strategy (zero on-device transposes):
  - host feeds rgb^T (d padded 400->512), pose^T in fp8e4m3 so projections
    contract d on partitions with DoubleRow (2 MACs/cell/cycle)
  - qT,kT computed h-major [h,t]; scores computed transposed ST=[tk,tq]
  - exp(ST - ln 32) on ACT (the 1/32 keeps unnormalized O inside fp8e4m3
    range; it cancels in the normalization); O^T accumulated via lhsT=v
    (natural layout), rhs=exp(ST); row-sums via a ones-vector matmul
  - every matmul uses fp8e4m3 operands with perf_mode=DoubleRow (pairs of
    128-row k-subtiles), fp32 PSUM accumulation everywhere
  - device returns unnormalized (A@v)@Wp and rowsum(A); the host applies
    y = rgb + bp' + yun/sums in fp32 (0.1% of the FLOPs, exact division)
"""

import sys

if "/opt/trn_rl_repo" not in sys.path:
    sys.path.insert(0, "/opt/trn_rl_repo")

from contextlib import ExitStack

import ml_dtypes
import numpy as np

import concourse.mybir as mybir
import concourse.tile as tile
from concourse import bacc, bass_utils

FP8 = mybir.dt.float8e4
F32 = mybir.dt.float32
NP_FP8 = ml_dtypes.float8_e4m3

B, T, DR, DP, H = 8, 2048, 400, 256, 512
PART = 128
DRP = 512                # rgb feature dim padded to 4*128
TQC = 512                # tq chunk width (max PSUM free dim)
NCH = T // TQC           # 4 chunks
NTK = T // PART          # 16 key tiles
NKP = NTK // 2           # 8 key tile pairs (DoubleRow)
NHT = H // PART          # 4 h tiles
NHP = NHT // 2           # 2 h tile pairs
NDR = DRP // PART        # 4 padded-rgb d tiles
NDRP = NDR // 2          # 2 pairs
NDP = DP // PART         # 2 pose d tiles
SCALE = float(1.0 / np.sqrt(np.float32(H)))
EXP_BIAS = float(-np.log(32.0))

AT = mybir.ActivationFunctionType
OP = mybir.AluOpType
DRM = mybir.MatmulPerfMode.DoubleRow


def build_nc():
    nc = bacc.Bacc(
        "TRN2",
        target_bir_lowering=False,
        debug=False,
        enable_asserts=False,
        num_devices=8,
    )
    xT = nc.dram_tensor("xT", (DRP, T), FP8, kind="ExternalInput").ap()
    pT = nc.dram_tensor("pT", (DP, T), FP8, kind="ExternalInput").ap()
    wq = nc.dram_tensor("wq", (DRP, H), FP8, kind="ExternalInput").ap()
    wk = nc.dram_tensor("wk", (DP, H), FP8, kind="ExternalInput").ap()
    wv = nc.dram_tensor("wv", (DP, H), FP8, kind="ExternalInput").ap()
    wp = nc.dram_tensor("wp", (H, DR), FP8, kind="ExternalInput").ap()
    bqc = nc.dram_tensor("bqc", (PART, NHT), F32, kind="ExternalInput").ap()
    bkc = nc.dram_tensor("bkc", (PART, NHT), F32, kind="ExternalInput").ap()
    yun = nc.dram_tensor("yun", (T, DR), F32, kind="ExternalOutput").ap()
    sums_out = nc.dram_tensor("sums_out", (NCH, TQC), F32, kind="ExternalOutput").ap()

    with tile.TileContext(nc) as tc, ExitStack() as ctx:
        const = ctx.enter_context(tc.tile_pool(name="const", bufs=1))
        mm_ps = ctx.enter_context(tc.tile_pool(name="mm_ps", bufs=3, space="PSUM"))
        ot_ps = ctx.enter_context(tc.tile_pool(name="ot_ps", bufs=4, space="PSUM"))
        sum_ps = ctx.enter_context(tc.tile_pool(name="sum_ps", bufs=1, space="PSUM"))
        ex_pool = ctx.enter_context(tc.tile_pool(name="ex_pool", bufs=10))
        sums_pool = ctx.enter_context(tc.tile_pool(name="sums_pool", bufs=2))
        ysb_pool = ctx.enter_context(tc.tile_pool(name="ysb_pool", bufs=4))

        # ---- persistent inputs ----
        # big streams on the sync HWDGE queue; small weights/biases in
        # parallel on the scalar HWDGE queue, kT path first on both
        wk8 = const.tile([PART, NDP, H], FP8, name="wk8")
        nc.scalar.dma_start(wk8[:], wk.rearrange("(k p) h -> p k h", p=PART))
        wv8 = const.tile([PART, NDP, H], FP8, name="wv8")
        nc.scalar.dma_start(wv8[:], wv.rearrange("(k p) h -> p k h", p=PART))
        wq8 = const.tile([PART, NDR, H], FP8, name="wq8")
        nc.scalar.dma_start(wq8[:], wq.rearrange("(k p) h -> p k h", p=PART))
        wp8 = const.tile([PART, NHT, DR], FP8, name="wp8")
        nc.scalar.dma_start(wp8[:], wp.rearrange("(k p) d -> p k d", p=PART))
        # descriptor-heavy tiny loads go on the otherwise-idle gpsimd queue
        bq_sb = const.tile([PART, NHT], F32, name="bq_sb")
        nc.gpsimd.dma_start(bq_sb[:], bqc[:])
        bk_sb = const.tile([PART, NHT], F32, name="bk_sb")
        nc.gpsimd.dma_start(bk_sb[:], bkc[:])
        p8 = const.tile([PART, NDP, T], FP8, name="p8")
        for h in range(4):  # split for earlier first-chunk availability
            nc.sync.dma_start(
                p8[:, :, h * (T // 4) : (h + 1) * (T // 4)],
                pT[:, h * (T // 4) : (h + 1) * (T // 4)].rearrange(
                    "(k p) t -> p k t", p=PART
                ),
            )
        x8 = const.tile([PART, NDR, T], FP8, name="x8")
        for h, eng in ((0, nc.sync), (1, nc.scalar)):
            eng.dma_start(
                x8[:, :, h * (T // 2) : (h + 1) * (T // 2)],
                xT[:, h * (T // 2) : (h + 1) * (T // 2)].rearrange(
                    "(k p) t -> p k t", p=PART
                ),
            )
        ones8 = const.tile([PART, 2, 16], FP8, name="ones8")
        nc.vector.memset(ones8[:], 1.0)
        expb = const.tile([PART, 1], F32, name="expb")
        nc.vector.memset(expb[:], EXP_BIAS)

        # ---- persistent intermediates (fp8 DoubleRow pair layouts) ----
        # qT8[i2][p, s, t] = q[h = i2*256 + s*128 + p, t]
        qT8 = [const.tile([PART, 2, T], FP8, name=f"qT8_{i}") for i in range(NHP)]
        kT8 = [const.tile([PART, 2, T], FP8, name=f"kT8_{i}") for i in range(NHP)]
        # v8[j2][p, s, h] = v[t = j2*256 + s*128 + p, h]
        v8 = [const.tile([PART, 2, H], FP8, name=f"v8_{j}") for j in range(NKP)]
        # ot8[i2][p, s, t] = O[h = i2*256 + s*128 + p, t] (unnormalized, /32)
        ot8 = [const.tile([PART, 2, T], FP8, name=f"ot8_{i}") for i in range(NHP)]

        def evict_biased(n, dst, ps, bias_ap, scale):
            """PSUM->SBUF cast with scale*x+bias, alternating DVE/ACT."""
            if n % 2 == 0:
                if scale == 1.0:
                    nc.vector.tensor_scalar_add(dst, ps, bias_ap)
                else:
                    nc.vector.tensor_scalar(
                        dst, ps, scale, bias_ap, op0=OP.mult, op1=OP.add
                    )
            else:
                nc.scalar.activation(dst, ps, AT.Identity, bias=bias_ap, scale=scale)

        # ---- phase B: projections (all DoubleRow over d pairs) ----
        # kT[h,t] = (Wk[d,h].T @ pT[d,t]) * scale + bk*scale -> fp8
        # (c-outer: chunks 0/1 only need the first half of p8)
        for c in range(NCH):
            for i in range(NHT):
                ps = mm_ps.tile([PART, TQC], F32, name=f"kps_{i}_{c}", tag="mmps")
                nc.tensor.matmul(
                    ps[:],
                    wk8[:, :, i * PART : (i + 1) * PART],
                    p8[:, :, c * TQC : (c + 1) * TQC],
                    start=True,
                    stop=True,
                    perf_mode=DRM,
                )
                evict_biased(
                    c * NHT + i,
                    kT8[i // 2][:, i % 2, c * TQC : (c + 1) * TQC],
                    ps[:],
                    bk_sb[:, i : i + 1],
                    SCALE,
                )
        # v[t,h] = pT[d,t].T @ Wv[d,h] -> fp8
        for j in range(NTK):
            ps = mm_ps.tile([PART, H], F32, name=f"vps_{j}", tag="mmps")
            nc.tensor.matmul(
                ps[:],
                p8[:, :, j * PART : (j + 1) * PART],
                wv8[:],
                start=True,
                stop=True,
                perf_mode=DRM,
            )
            if j % 2 == 0:
                nc.scalar.copy(v8[j // 2][:, j % 2, :], ps[:])
            else:
                nc.vector.tensor_copy(v8[j // 2][:, j % 2, :], ps[:])
        # qT[h,t] = Wq[d,h].T @ xT[d,t] + bq -> fp8 (c-outer so chunk 0 is ready early)
        for c in range(NCH):
            for i in range(NHT):
                ps = mm_ps.tile([PART, TQC], F32, name=f"qps_{i}_{c}", tag="mmps")
                for d2 in range(NDRP):
                    nc.tensor.matmul(
                        ps[:],
                        wq8[:, 2 * d2 : 2 * d2 + 2, i * PART : (i + 1) * PART],
                        x8[:, 2 * d2 : 2 * d2 + 2, c * TQC : (c + 1) * TQC],
                        start=(d2 == 0),
                        stop=(d2 == NDRP - 1),
                        perf_mode=DRM,
                    )
                evict_biased(
                    c * NHT + i + 1,
                    qT8[i // 2][:, i % 2, c * TQC : (c + 1) * TQC],
                    ps[:],
                    bq_sb[:, i : i + 1],
                    1.0,
                )

        # ---- phase C: attention, chunked over tq ----
        # phase D (output projection) for chunk c-1 is emitted a few j-steps
        # into chunk c so its PSUM/engine traffic doesn't cluster at the
        # chunk boundary.
        def emit_y_tile(c, tl):
            tg = c * (TQC // PART) + tl
            yp = mm_ps.tile([PART, DR], F32, name=f"yp_{tg}", tag="mmps")
            for i2 in range(NHP):
                nc.tensor.matmul(
                    yp[:],
                    ot8[i2][:, :, tg * PART : (tg + 1) * PART],
                    wp8[:, 2 * i2 : 2 * i2 + 2, :],
                    start=(i2 == 0),
                    stop=(i2 == NHP - 1),
                    perf_mode=DRM,
                )
            ysb = ysb_pool.tile([PART, DR], F32, name=f"ysb_{tg}", tag="ysb")
            # alternate engines so the final chain parallelizes at kernel end
            if tl % 2 == 0:
                nc.vector.tensor_copy(ysb[:], yp[:])
                nc.sync.dma_start(yun[tg * PART : (tg + 1) * PART, :], ysb[:])
            else:
                nc.scalar.copy(ysb[:], yp[:])
                nc.gpsimd.dma_start(yun[tg * PART : (tg + 1) * PART, :], ysb[:])

        for c in range(NCH):
            otps = [
                ot_ps.tile([PART, TQC], F32, name=f"otp_{c}_{i}", tag="otp")
                for i in range(NHT)
            ]
            sps = sum_ps.tile([1, TQC], F32, name=f"sump_{c}", tag="sump")
            exs = []
            for j in range(NTK):

                st = mm_ps.tile([PART, TQC], F32, name=f"st_{c}_{j}", tag="mmps")
                for i2 in range(NHP):
                    nc.tensor.matmul(
                        st[:],
                        kT8[i2][:, :, j * PART : (j + 1) * PART],
                        qT8[i2][:, :, c * TQC : (c + 1) * TQC],
                        start=(i2 == 0),
                        stop=(i2 == NHP - 1),
                        perf_mode=DRM,
                    )
                if j % 2 == 0:
                    ex = ex_pool.tile([PART, 2, TQC], FP8, name=f"ex_{c}_{j}", tag="ex")
                    exs.append(ex)
                nc.scalar.activation(
                    exs[-1][:, j % 2, :], st[:], AT.Exp, bias=expb[:]
                )
                # y tiles of the previous chunk, spread through this chunk so
                # they don't crowd the mmps PSUM slots at the boundary; the
                # last one fills the PE bubble while the final exp is on ACT
                if c > 0 and j in (5, 9, 13, 15):
                    emit_y_tile(c - 1, (5, 9, 13, 15).index(j))
                if j % 2 == 1:
                    j2 = j // 2
                    ex = exs[-1]
                    # sums first: its 2-column LDWEIGHTS gives the weight-load
                    # port slack between the 256-column v-slice loads
                    nc.tensor.matmul(
                        sps[:],
                        ones8[:, :, 0:1],
                        ex[:],
                        start=(j2 == 0),
                        stop=(j2 == NKP - 1),
                        perf_mode=DRM,
                    )
                    for i in range(NHT):
                        nc.tensor.matmul(
                            otps[i][:],
                            v8[j2][:, :, i * PART : (i + 1) * PART],
                            ex[:],
                            start=(j2 == 0),
                            stop=(j2 == NKP - 1),
                            perf_mode=DRM,
                        )
            for i in range(NHT):
                # split across ACT/DVE so neither engine bursts at the boundary
                dst = ot8[i // 2][:, i % 2, c * TQC : (c + 1) * TQC]
                if i % 2 == 0:
                    nc.scalar.copy(dst, otps[i][:])
                else:
                    nc.vector.tensor_copy(dst, otps[i][:])
            sums_sb = sums_pool.tile([1, TQC], F32, name=f"sums_{c}", tag="sums")
            nc.vector.tensor_copy(sums_sb[:], sps[:])
            nc.sync.dma_start(sums_out[c : c + 1, :], sums_sb[:])

        for tl in range(TQC // PART):
            emit_y_tile(NCH - 1, tl)

    nc.compile()
    return nc


_NC_CACHE = None


def get_nc():
    global _NC_CACHE
    if _NC_CACHE is None:
        _NC_CACHE = build_nc()
    return _NC_CACHE


def make_in_maps(rgb, pose, Wq, bq, Wk, bk, Wv, bv, Wp, bp):
    rgb = np.asarray(rgb, np.float32)
    pose = np.asarray(pose, np.float32)
    Wq, bq = np.asarray(Wq, np.float32), np.asarray(bq, np.float32)
    Wk, bk = np.asarray(Wk, np.float32), np.asarray(bk, np.float32)
    Wv = np.asarray(Wv, np.float32)
    Wp = np.asarray(Wp, np.float32)

    xT = np.zeros((B, DRP, T), NP_FP8)
    xT[:, :DR, :] = np.swapaxes(rgb, 1, 2).astype(NP_FP8)
    pT = np.ascontiguousarray(np.swapaxes(pose, 1, 2)).astype(NP_FP8)
    wq8 = np.zeros((DRP, H), NP_FP8)
    wq8[:DR] = Wq.astype(NP_FP8)
    wk8 = Wk.astype(NP_FP8)
    wv8 = Wv.astype(NP_FP8)
    wp8 = Wp.astype(NP_FP8)
    bqc = np.ascontiguousarray(bq.reshape(NHT, PART).T).astype(np.float32)
    bkc = np.ascontiguousarray((bk * SCALE).reshape(NHT, PART).T).astype(np.float32)
    return [
        dict(
            xT=xT[b], pT=pT[b],
            wq=wq8, wk=wk8, wv=wv8, wp=wp8, bqc=bqc, bkc=bkc,
        )
        for b in range(B)
    ]


def kernel(rgb, pose, Wq, bq, Wk, bk, Wv, bv, Wp, bp):
    rgb = np.asarray(rgb, np.float32)
    Wp_f = np.asarray(Wp, np.float32)
    bp_eff = np.asarray(bp, np.float32) + np.asarray(bv, np.float32) @ Wp_f
    in_maps = make_in_maps(rgb, pose, Wq, bq, Wk, bk, Wv, bv, Wp, bp)
    res = bass_utils.run_bass_kernel_spmd(get_nc(), in_maps, core_ids=list(range(B)))
    out = np.empty((B, T, DR), np.float32)
    for b in range(B):
        yun = res.results[b]["yun"]
        sums = res.results[b]["sums_out"].reshape(T)
        out[b] = rgb[b] + bp_eff + yun / sums[:, None]
    return out


# revision 29
# speedup vs baseline: 1.0196x; 1.0196x over previous
"""Trainium2 Bass kernel for nn_CrossModalFusion (single-head cross attention).

Per-batch-element cross attention, data-parallel over B=8 across 8 NeuronCores.

Per core (T=2048, D_RGB=400, D_POSE=256, H=512):
    q = rgb @ Wq + bq ; k = pose @ Wk + bk ; v = pose @ Wv
    S = q @ k.T / sqrt(H) ; A = exp(S) (no max-sub needed; scores are O(1))
    y = rgb + bp + bv@Wp + (A @ v) @ Wp / rowsum(A)

Layout strategy (zero on-device transposes):
  - host feeds rgb^T (d padded 400->512), pose^T in fp8e4m3 so projections
    contract d on partitions with DoubleRow (2 MACs/cell/cycle)
  - qT,kT computed h-major [h,t]; scores computed transposed ST=[tk,tq]
  - exp(ST - ln 32) on ACT (the 1/32 keeps unnormalized O inside fp8e4m3
    range; it cancels in the normalization); O^T accumulated via lhsT=v
    (natural layout), rhs=exp(ST); row-sums via a ones-vector matmul
  - every matmul uses fp8e4m3 operands with perf_mode=DoubleRow (pairs of
    128-row k-subtiles), fp32 PSUM accumulation everywhere
  - device returns unnormalized (A@v)@Wp and rowsum(A); the host applies
    y = rgb + bp' + yun/sums in fp32 (0.1% of the FLOPs, exact division)
"""

import sys

if "/opt/trn_rl_repo" not in sys.path:
    sys.path.insert(0, "/opt/trn_rl_repo")

from contextlib import ExitStack

import ml_dtypes
import numpy as np

import concourse.mybir as mybir
import concourse.tile as tile
from concourse import bacc, bass_utils

FP8 = mybir.dt.float8e4
F32 = mybir.dt.float32
NP_FP8 = ml_dtypes.float8_e4m3

B, T, DR, DP, H = 8, 2048, 400, 256, 512
PART = 128
DRP = 512                # rgb feature dim padded to 4*128
TQC = 512                # tq chunk width (max PSUM free dim)
NCH = T // TQC           # 4 chunks
NTK = T // PART          # 16 key tiles
NKP = NTK // 2           # 8 key tile pairs (DoubleRow)
NHT = H // PART          # 4 h tiles
NHP = NHT // 2           # 2 h tile pairs
NDR = DRP // PART        # 4 padded-rgb d tiles
NDRP = NDR // 2          # 2 pairs
NDP = DP // PART         # 2 pose d tiles
SCALE = float(1.0 / np.sqrt(np.float32(H)))
EXP_BIAS = float(-np.log(32.0))

AT = mybir.ActivationFunctionType
OP = mybir.AluOpType
DRM = mybir.MatmulPerfMode.DoubleRow


def build_nc():
    nc = bacc.Bacc(
        "TRN2",
        target_bir_lowering=False,
        debug=False,
        enable_asserts=False,
        num_devices=8,
    )
    xT = nc.dram_tensor("xT", (DRP, T), FP8, kind="ExternalInput").ap()
    pT = nc.dram_tensor("pT", (DP, T), FP8, kind="ExternalInput").ap()
    wq = nc.dram_tensor("wq", (DRP, H), FP8, kind="ExternalInput").ap()
    wk = nc.dram_tensor("wk", (DP, H), FP8, kind="ExternalInput").ap()
    wv = nc.dram_tensor("wv", (DP, H), FP8, kind="ExternalInput").ap()
    wp = nc.dram_tensor("wp", (H, DR), FP8, kind="ExternalInput").ap()
    bqc = nc.dram_tensor("bqc", (PART, NHT), F32, kind="ExternalInput").ap()
    bkc = nc.dram_tensor("bkc", (PART, NHT), F32, kind="ExternalInput").ap()
    yun = nc.dram_tensor("yun", (T, DR), F32, kind="ExternalOutput").ap()
    sums_out = nc.dram_tensor("sums_out", (NCH, TQC), F32, kind="ExternalOutput").ap()

    with tile.TileContext(nc) as tc, ExitStack() as ctx:
        const = ctx.enter_context(tc.tile_pool(name="const", bufs=1))
        mm_ps = ctx.enter_context(tc.tile_pool(name="mm_ps", bufs=3, space="PSUM"))
        ot_ps = ctx.enter_context(tc.tile_pool(name="ot_ps", bufs=4, space="PSUM"))
        sum_ps = ctx.enter_context(tc.tile_pool(name="sum_ps", bufs=1, space="PSUM"))
        ex_pool = ctx.enter_context(tc.tile_pool(name="ex_pool", bufs=10))
        sums_pool = ctx.enter_context(tc.tile_pool(name="sums_pool", bufs=2))
        ysb_pool = ctx.enter_context(tc.tile_pool(name="ysb_pool", bufs=4))

        # ---- persistent inputs ----
        # big streams on the sync HWDGE queue; small weights/biases in
        # parallel on the scalar HWDGE queue, kT path first on both
        wk8 = const.tile([PART, NDP, H], FP8, name="wk8")
        nc.scalar.dma_start(wk8[:], wk.rearrange("(k p) h -> p k h", p=PART))
        wv8 = const.tile([PART, NDP, H], FP8, name="wv8")
        nc.scalar.dma_start(wv8[:], wv.rearrange("(k p) h -> p k h", p=PART))
        wq8 = const.tile([PART, NDR, H], FP8, name="wq8")
        nc.scalar.dma_start(wq8[:], wq.rearrange("(k p) h -> p k h", p=PART))
        wp8 = const.tile([PART, NHT, DR], FP8, name="wp8")
        nc.scalar.dma_start(wp8[:], wp.rearrange("(k p) d -> p k d", p=PART))
        # descriptor-heavy tiny loads go on the otherwise-idle gpsimd queue
        bq_sb = const.tile([PART, NHT], F32, name="bq_sb")
        nc.gpsimd.dma_start(bq_sb[:], bqc[:])
        bk_sb = const.tile([PART, NHT], F32, name="bk_sb")
        nc.gpsimd.dma_start(bk_sb[:], bkc[:])
        p8 = const.tile([PART, NDP, T], FP8, name="p8")
        for h in range(4):  # split for earlier first-chunk availability
            nc.sync.dma_start(
                p8[:, :, h * (T // 4) : (h + 1) * (T // 4)],
                pT[:, h * (T // 4) : (h + 1) * (T // 4)].rearrange(
                    "(k p) t -> p k t", p=PART
                ),
            )
        x8 = const.tile([PART, NDR, T], FP8, name="x8")
        for h, eng in ((0, nc.sync), (1, nc.scalar)):
            eng.dma_start(
                x8[:, :, h * (T // 2) : (h + 1) * (T // 2)],
                xT[:, h * (T // 2) : (h + 1) * (T // 2)].rearrange(
                    "(k p) t -> p k t", p=PART
                ),
            )
        ones8 = const.tile([PART, 2, 16], FP8, name="ones8")
        nc.vector.memset(ones8[:], 1.0)
        expb = const.tile([PART, 1], F32, name="expb")
        nc.vector.memset(expb[:], EXP_BIAS)

        # ---- persistent intermediates (fp8 DoubleRow pair layouts) ----
        # qT8[i2][p, s, t] = q[h = i2*256 + s*128 + p, t]
        qT8 = [const.tile([PART, 2, T], FP8, name=f"qT8_{i}") for i in range(NHP)]
        kT8 = [const.tile([PART, 2, T], FP8, name=f"kT8_{i}") for i in range(NHP)]
        # v8[j2][p, s, h] = v[t = j2*256 + s*128 + p, h]
        v8 = [const.tile([PART, 2, H], FP8, name=f"v8_{j}") for j in range(NKP)]
        # ot8[i2][p, s, t] = O[h = i2*256 + s*128 + p, t] (unnormalized, /32)
        ot8 = [const.tile([PART, 2, T], FP8, name=f"ot8_{i}") for i in range(NHP)]

        def evict_biased(n, dst, ps, bias_ap, scale):
            """PSUM->SBUF cast with scale*x+bias, alternating DVE/ACT."""
            if n % 2 == 0:
                if scale == 1.0:
                    nc.vector.tensor_scalar_add(dst, ps, bias_ap)
                else:
                    nc.vector.tensor_scalar(
                        dst, ps, scale, bias_ap, op0=OP.mult, op1=OP.add
                    )
            else:
                nc.scalar.activation(dst, ps, AT.Identity, bias=bias_ap, scale=scale)

        # ---- phase B: projections (all DoubleRow over d pairs) ----
        # kT[h,t] = (Wk[d,h].T @ pT[d,t]) * scale + bk*scale -> fp8
        # (c-outer: chunks 0/1 only need the first half of p8)
        for c in range(NCH):
            for i in range(NHT):
                ps = mm_ps.tile([PART, TQC], F32, name=f"kps_{i}_{c}", tag="mmps")
                nc.tensor.matmul(
                    ps[:],
                    wk8[:, :, i * PART : (i + 1) * PART],
                    p8[:, :, c * TQC : (c + 1) * TQC],
                    start=True,
                    stop=True,
                    perf_mode=DRM,
                )
                evict_biased(
                    c * NHT + i,
                    kT8[i // 2][:, i % 2, c * TQC : (c + 1) * TQC],
                    ps[:],
                    bk_sb[:, i : i + 1],
                    SCALE,
                )
        # v[t,h] = pT[d,t].T @ Wv[d,h] -> fp8
        for j in range(NTK):
            ps = mm_ps.tile([PART, H], F32, name=f"vps_{j}", tag="mmps")
            nc.tensor.matmul(
                ps[:],
                p8[:, :, j * PART : (j + 1) * PART],
                wv8[:],
                start=True,
                stop=True,
                perf_mode=DRM,
            )
            if j % 2 == 0:
                nc.scalar.copy(v8[j // 2][:, j % 2, :], ps[:])
            else:
                nc.vector.tensor_copy(v8[j // 2][:, j % 2, :], ps[:])
        # qT[h,t] = Wq[d,h].T @ xT[d,t] + bq -> fp8 (c-outer so chunk 0 is ready early)
        for c in range(NCH):
            for i in range(NHT):
                ps = mm_ps.tile([PART, TQC], F32, name=f"qps_{i}_{c}", tag="mmps")
                for d2 in range(NDRP):
                    nc.tensor.matmul(
                        ps[:],
                        wq8[:, 2 * d2 : 2 * d2 + 2, i * PART : (i + 1) * PART],
                        x8[:, 2 * d2 : 2 * d2 + 2, c * TQC : (c + 1) * TQC],
                        start=(d2 == 0),
                        stop=(d2 == NDRP - 1),
                        perf_mode=DRM,
                    )
                evict_biased(
                    c * NHT + i + 1,
                    qT8[i // 2][:, i % 2, c * TQC : (c + 1) * TQC],
                    ps[:],
                    bq_sb[:, i : i + 1],
                    1.0,
                )

        # ---- phase C: attention, chunked over tq ----
        # phase D (output projection) for chunk c-1 is emitted a few j-steps
        # into chunk c so its PSUM/engine traffic doesn't cluster at the
        # chunk boundary.
        def emit_y_tile(c, tl):
            tg = c * (TQC // PART) + tl
            yp = mm_ps.tile([PART, DR], F32, name=f"yp_{tg}", tag="mmps")
            for i2 in range(NHP):
                nc.tensor.matmul(
                    yp[:],
                    ot8[i2][:, :, tg * PART : (tg + 1) * PART],
                    wp8[:, 2 * i2 : 2 * i2 + 2, :],
                    start=(i2 == 0),
                    stop=(i2 == NHP - 1),
                    perf_mode=DRM,
                )
            ysb = ysb_pool.tile([PART, DR], F32, name=f"ysb_{tg}", tag="ysb")
            # alternate engines so the final chain parallelizes at kernel end
            if tl % 2 == 0:
                nc.vector.tensor_copy(ysb[:], yp[:])
                nc.sync.dma_start(yun[tg * PART : (tg + 1) * PART, :], ysb[:])
            else:
                nc.scalar.copy(ysb[:], yp[:])
                nc.scalar.dma_start(yun[tg * PART : (tg + 1) * PART, :], ysb[:])

        for c in range(NCH):
            otps = [
                ot_ps.tile([PART, TQC], F32, name=f"otp_{c}_{i}", tag="otp")
                for i in range(NHT)
            ]
            sps = sum_ps.tile([1, TQC], F32, name=f"sump_{c}", tag="sump")
            exs = []
            for j in range(NTK):

                st = mm_ps.tile([PART, TQC], F32, name=f"st_{c}_{j}", tag="mmps")
                for i2 in range(NHP):
                    nc.tensor.matmul(
                        st[:],
                        kT8[i2][:, :, j * PART : (j + 1) * PART],
                        qT8[i2][:, :, c * TQC : (c + 1) * TQC],
                        start=(i2 == 0),
                        stop=(i2 == NHP - 1),
                        perf_mode=DRM,
                    )
                if j % 2 == 0:
                    ex = ex_pool.tile([PART, 2, TQC], FP8, name=f"ex_{c}_{j}", tag="ex")
                    exs.append(ex)
                nc.scalar.activation(
                    exs[-1][:, j % 2, :], st[:], AT.Exp, bias=expb[:]
                )
                # y tiles of the previous chunk, spread through this chunk so
                # they don't crowd the mmps PSUM slots at the boundary; the
                # last one fills the PE bubble while the final exp is on ACT
                if c > 0 and j in (5, 9, 13, 15):
                    emit_y_tile(c - 1, (5, 9, 13, 15).index(j))
                if j % 2 == 1:
                    j2 = j // 2
                    ex = exs[-1]
                    # sums first: its 2-column LDWEIGHTS gives the weight-load
                    # port slack between the 256-column v-slice loads
                    nc.tensor.matmul(
                        sps[:],
                        ones8[:, :, 0:1],
                        ex[:],
                        start=(j2 == 0),
                        stop=(j2 == NKP - 1),
                        perf_mode=DRM,
                    )
                    for i in range(NHT):
                        nc.tensor.matmul(
                            otps[i][:],
                            v8[j2][:, :, i * PART : (i + 1) * PART],
                            ex[:],
                            start=(j2 == 0),
                            stop=(j2 == NKP - 1),
                            perf_mode=DRM,
                        )
            for i in range(NHT):
                # split across ACT/DVE so neither engine bursts at the boundary
                dst = ot8[i // 2][:, i % 2, c * TQC : (c + 1) * TQC]
                if i % 2 == 0:
                    nc.scalar.copy(dst, otps[i][:])
                else:
                    nc.vector.tensor_copy(dst, otps[i][:])
            sums_sb = sums_pool.tile([1, TQC], F32, name=f"sums_{c}", tag="sums")
            nc.vector.tensor_copy(sums_sb[:], sps[:])
            nc.sync.dma_start(sums_out[c : c + 1, :], sums_sb[:])

        for tl in range(TQC // PART):
            emit_y_tile(NCH - 1, tl)

    nc.compile()
    return nc


_NC_CACHE = None


def get_nc():
    global _NC_CACHE
    if _NC_CACHE is None:
        _NC_CACHE = build_nc()
    return _NC_CACHE


def make_in_maps(rgb, pose, Wq, bq, Wk, bk, Wv, bv, Wp, bp):
    rgb = np.asarray(rgb, np.float32)
    pose = np.asarray(pose, np.float32)
    Wq, bq = np.asarray(Wq, np.float32), np.asarray(bq, np.float32)
    Wk, bk = np.asarray(Wk, np.float32), np.asarray(bk, np.float32)
    Wv = np.asarray(Wv, np.float32)
    Wp = np.asarray(Wp, np.float32)

    xT = np.zeros((B, DRP, T), NP_FP8)
    xT[:, :DR, :] = np.swapaxes(rgb, 1, 2).astype(NP_FP8)
    pT = np.ascontiguousarray(np.swapaxes(pose, 1, 2)).astype(NP_FP8)
    wq8 = np.zeros((DRP, H), NP_FP8)
    wq8[:DR] = Wq.astype(NP_FP8)
    wk8 = Wk.astype(NP_FP8)
    wv8 = Wv.astype(NP_FP8)
    wp8 = Wp.astype(NP_FP8)
    bqc = np.ascontiguousarray(bq.reshape(NHT, PART).T).astype(np.float32)
    bkc = np.ascontiguousarray((bk * SCALE).reshape(NHT, PART).T).astype(np.float32)
    return [
        dict(
            xT=xT[b], pT=pT[b],
            wq=wq8, wk=wk8, wv=wv8, wp=wp8, bqc=bqc, bkc=bkc,
        )
        for b in range(B)
    ]


def kernel(rgb, pose, Wq, bq, Wk, bk, Wv, bv, Wp, bp):
    rgb = np.asarray(rgb, np.float32)
    Wp_f = np.asarray(Wp, np.float32)
    bp_eff = np.asarray(bp, np.float32) + np.asarray(bv, np.float32) @ Wp_f
    in_maps = make_in_maps(rgb, pose, Wq, bq, Wk, bk, Wv, bv, Wp, bp)
    res = bass_utils.run_bass_kernel_spmd(get_nc(), in_maps, core_ids=list(range(B)))
    out = np.empty((B, T, DR), np.float32)
    for b in range(B):
        yun = res.results[b]["yun"]
        sums = res.results[b]["sums_out"].reshape(T)
        out[b] = rgb[b] + bp_eff + yun / sums[:, None]
    return out
